# revision 19
# baseline (speedup 1.0000x reference)
"""BetaGNN message-passing kernel for 8 Trainium2 NeuronCores.

Strategy (dest-row sharding, 6250 nodes/core):
  - Host relabels nodes: sorted by in-degree, dealt round-robin to cores so
    every core's tile t has near-identical max-degree -> uniform chunk counts.
  - Hop 1 (AH = A @ relu(x @ W_in^T + b)): no gather. Host pre-gathers the
    3-wide input features per edge (plus a ones column); the PE recomputes h
    per edge-slot, TWO chunks per matmul (K=8 block-diagonal W_in, N=512).
    Edge values (x16) fold into the relu via per-partition scale; fp8
    messages accumulate FOUR chunks per DoubleRow identity matmul into a
    split [128,512] accumulator whose halves are summed in the epilogue.
  - Local AH rows (x16, fp8) are AllGathered in THREE slices, each fired as
    soon as its tiles finish so collectives overlap hop-1 compute and the
    early hop-2 gathers. Each slice lands in a compact table so gather
    indices stay int16.
  - Hop 2 (A2H = A @ AH): edges are bucketed by source slice and packed
    128/chunk with a general scatter matrix S (fp8, x16) routing
    slot -> dest row. Rows are dma_gathered (256B fp8) on 4 SWDGE queues;
    pairs of chunks accumulate with one DoubleRow matmul. Buckets are
    processed in separate passes (bf16 partials staged in SBUF) so a
    not-yet-ready collective never head-of-line blocks the gather queue;
    the next collective's dispatch is emitted in the middle of the previous
    bucket's gather stream.
  - Dense tail in transposed layout: AH/A2H tiles transpose via fp8 matmuls
    against scaled identities into [128, 2, NPAD] fp8 residents;
    h2^T = relu(W1 AH^T + W2 A2H^T) (DoubleRow over the two hid halves) and
    g = softplus(W_out h2^T + b_out), one 512-col block at a time,
    interleaved into the last hop-2 pass. All fp8 scale factors are powers
    of two (exact).
"""

import sys

for _p in ("/opt/trn_rl_repo", "/root/.axon_site/_ro/trn_rl_repo"):
    if _p not in sys.path:
        sys.path.insert(0, _p)

import numpy as np
import ml_dtypes

import concourse.bacc as bacc
import concourse.bass as bass
import concourse.mybir as mybir
from concourse import tile
from concourse.bass_utils import run_bass_kernel_spmd

F32 = mybir.dt.float32
F32R = mybir.dt.float32r
BF16 = mybir.dt.bfloat16
FP8 = mybir.dt.float8e4
I16 = mybir.dt.int16
AF = mybir.ActivationFunctionType
DR = mybir.MatmulPerfMode.DoubleRow
NPFP8 = ml_dtypes.float8_e4m3fn

MAX_CALL_CHUNKS = 12      # <=12 chunks (1536 idxs) per dma_gather call
NQUEUES = 4               # SWDGE queues for gather concurrency
COLL_FRAC = 0.35          # emit next collective after this fraction of calls


class Cfg:
    def __init__(self, P, E, nc=8, hid=256):
        assert P % (nc * 2) == 0
        self.P, self.E, self.NC, self.HID = P, E, nc, hid
        self.NPC = P // nc                    # nodes per core
        self.NT = (self.NPC + 127) // 128     # dest tiles per core
        self.NPAD = self.NT * 128
        if self.NT > 40:
            self.SPLITS = [14, 28, 42]        # bucket boundaries (tiles)
        else:
            self.SPLITS = [max(1, self.NT // 2)]
        bounds = [0] + self.SPLITS + [self.NT]
        self.NB = len(bounds) - 1
        self.BROWS = []                       # locals per bucket
        for i in range(self.NB):
            lo = bounds[i] * 128
            hi = min(bounds[i + 1] * 128, self.NPC)
            self.BROWS.append(hi - lo)
        self.BT = bounds                      # tile bounds per bucket
        self.BLK = []
        off = 0
        while off < self.NPAD:
            w = min(512, self.NPAD - off)
            self.BLK.append((off, w))
            off += w


def _plan(cfg, deg):
    """Hop-1 plan: chunk count per tile = max in-degree in the tile."""
    P, NC, NT = cfg.P, cfg.NC, cfg.NT
    order = np.argsort(-deg, kind="stable")
    rank = np.empty(P, np.int64)
    rank[order] = np.arange(P)
    core_of = rank % NC
    local_of = rank // NC
    gid = core_of * cfg.NPC + local_of
    degs_sorted = deg[order]
    NCHUNK = []
    for t in range(NT):
        NCHUNK.append(max(1, int(degs_sorted[min(t * 128 * NC, P - 1)])))
    NCHUNK = np.array(NCHUNK, np.int64)
    tile_off = np.concatenate([[0], np.cumsum(NCHUNK)])
    return core_of, local_of, gid, NCHUNK, tile_off, int(tile_off[-1])


def _split_calls(nchunks):
    """Split a chunk count into gather calls <= MAX_CALL_CHUNKS, keeping
    every non-final call even so DoubleRow pairs never straddle calls."""
    out = []
    rem = nchunks
    while rem:
        g = min(MAX_CALL_CHUNKS, rem)
        if g < rem and g % 2:
            g -= 1
        out.append(g)
        rem -= g
    return out


def _pack_pairs(x4T, lo, hi):
    """Pack chunks [lo,hi) of x4T ([4, TC*128]) in h-pair layout: pair p ->
    partitions 32*(p%4)+(0..8), col block p//4. Odd tail chunk packs alone
    in the A-half of its pair slot."""
    n = hi - lo
    npr = (n + 1) // 2
    NQ = (npr + 3) // 4
    x4q = np.zeros((128, NQ * 128), np.float32)
    for p in range(npr):
        j, q = p % 4, p // 4
        kA = lo + 2 * p
        x4q[32 * j:32 * j + 4, q * 128:(q + 1) * 128] = \
            x4T[:, kA * 128:(kA + 1) * 128]
        if 2 * p + 1 < n:
            kB = kA + 1
            x4q[32 * j + 4:32 * j + 8, q * 128:(q + 1) * 128] = \
                x4T[:, kB * 128:(kB + 1) * 128]
    return x4q, NQ


def _prepare(cfg, beta, degree, A_rows, A_cols, A_vals,
             W_in, b_in, W_mp1, W_mp2, W_out, b_out):
    P, E, NC, NPC, NT = cfg.P, cfg.E, cfg.NC, cfg.NPC, cfg.NT
    NB = cfg.NB
    deg = np.bincount(A_rows, minlength=P).astype(np.int64)
    core_of, local_of, gid, NCHUNK, tile_off, TC = _plan(cfg, deg)

    # ---- hop-1 edge slots (slot column == dest column) ----
    d_gid = gid[A_rows.astype(np.int64)]
    oe = np.argsort(d_gid, kind="stable")
    sd = d_gid[oe]
    first = np.r_[True, sd[1:] != sd[:-1]]
    cumstart = np.maximum.accumulate(np.where(first, np.arange(E), 0))
    chunk = np.arange(E) - cumstart
    e_core = sd // NPC
    e_local = sd % NPC
    e_col = e_local % 128
    e_k = tile_off[e_local // 128] + chunk
    e_slot = e_k * 128 + e_col
    src1 = A_cols.astype(np.int64)[oe]
    vals1 = A_vals[oe].astype(np.float32)

    x4_all = np.stack([beta[:, 0], beta[:, 0] ** 2, degree[:, 0],
                       np.ones(P, np.float32)], axis=0).astype(np.float32)

    # ---- hop-2 edge plan: sort by (core, tile, bucket) ----
    s_gid = gid[A_cols.astype(np.int64)]
    c2_core = d_gid // NPC
    c2_loc = d_gid % NPC
    c2_tile = c2_loc // 128
    c2_dcol = c2_loc % 128
    s_loc = s_gid % NPC
    s_core = s_gid // NPC
    blo = np.array([cfg.BT[i] * 128 for i in range(NB)], np.int64)
    c2_b = np.searchsorted(blo, s_loc, side="right") - 1
    brows = np.array(cfg.BROWS, np.int64)
    c2_tidx = s_core * brows[c2_b] + (s_loc - blo[c2_b])
    o2 = np.lexsort((c2_b, c2_tile, c2_core))
    g_core = c2_core[o2]
    g_tile = c2_tile[o2]
    g_b = c2_b[o2]
    g_dcol = c2_dcol[o2]
    g_tidx = c2_tidx[o2]
    g_val = A_vals[o2].astype(np.float32)
    key = (g_core * NT + g_tile) * NB + g_b
    kfirst = np.r_[True, key[1:] != key[:-1]]
    kcum = np.maximum.accumulate(np.where(kfirst, np.arange(E), 0))
    g_pos = np.arange(E) - kcum

    # shared SPMD structure (max over cores, min 1 chunk per (t,b))
    cnt_all = np.zeros((NC, NT, NB), np.int64)
    np.add.at(cnt_all, (g_core, g_tile, g_b), 1)
    nch = np.maximum(1, -(-cnt_all.max(axis=0) // 128))   # [NT, NB]
    flat = nch.reshape(-1)
    cbase = np.concatenate([[0], np.cumsum(flat)]).astype(np.int64)
    TOT = int(cbase[-1])
    # calls grouped bucket-major (pass order)
    calls = []          # (tile, bucket, chunk_base, g)
    for b in range(NB):
        for t in range(NT):
            base = int(cbase[t * NB + b])
            for g in _split_calls(int(nch[t, b])):
                calls.append((t, b, base, g))
                base += g
    NIC = sum(g * 128 // 16 for (_, _, _, g) in calls)

    part_bounds = [int(tile_off[bt]) for bt in cfg.BT]    # chunk bounds

    per_core = []
    for c in range(NC):
        # ---- hop 1 arrays ----
        m1 = e_core == c
        sl1 = e_slot[m1]
        x4T = np.zeros((4, TC * 128), np.float32)
        x4T[:, sl1] = x4_all[:, src1[m1]]
        v1 = np.zeros((128, TC), np.float32)
        v1[e_col[m1], e_k[m1]] = 16.0 * vals1[m1]
        xparts = []
        for i in range(NB):
            x4q, NQ = _pack_pairs(x4T, part_bounds[i], part_bounds[i + 1])
            xparts.append(x4q)

        # ---- hop 2 arrays ----
        m2 = g_core == c
        e_key = (g_tile[m2] * NB + g_b[m2])
        e_chunk = cbase[e_key] + g_pos[m2] // 128
        e_p = g_pos[m2] % 128
        S8 = np.zeros((128, TOT * 128), np.float32)
        S8[e_p, e_chunk * 128 + g_dcol[m2]] = 16.0 * g_val[m2]
        S8 = S8.astype(NPFP8)
        slot_idx = np.zeros(TOT * 128, np.int64)
        slot_idx[e_chunk * 128 + e_p] = g_tidx[m2]

        idxh = np.zeros((128, NIC), np.int16)
        col0 = 0
        for (t, b, base, g) in calls:
            ni = g * 128
            blockv = slot_idx[base * 128:base * 128 + ni].astype(np.int16)
            blockv = blockv.reshape(ni // 16, 16).T
            for q in range(8):
                idxh[16 * q:16 * (q + 1), col0:col0 + ni // 16] = blockv
            col0 += ni // 16
        pc = dict(v1=v1, s8=S8, idx=idxh)
        for i in range(NB):
            pc[f"x4_{i}"] = xparts[i]
        per_core.append(pc)

    # ---- constants (power-of-two scaled for fp8) ----
    wiT = np.concatenate([W_in.T.astype(np.float32),
                          b_in[None, :].astype(np.float32)], axis=0)
    HID = cfg.HID
    wiT2 = np.zeros((128, 2 * HID), np.float32)
    for j in range(4):
        wiT2[32 * j:32 * j + 4, 0:HID] = wiT
        wiT2[32 * j + 4:32 * j + 8, HID:2 * HID] = wiT

    def pack_w(W, scale):
        w = (scale * W.T.astype(np.float32)).reshape(2, 128, W.shape[0])
        return np.ascontiguousarray(w.transpose(1, 0, 2)).astype(NPFP8)

    idn2 = np.zeros((128, 2, 128), np.float32)
    idn2[np.arange(128), 0, np.arange(128)] = 1.0
    idn2[np.arange(128), 1, np.arange(128)] = 1.0

    consts = dict(
        wit2=wiT2,
        w1s=pack_w(W_mp1, 32.0),
        w2s=pack_w(W_mp2, 16.0),
        wos=(16.0 * W_out.reshape(2, 128).T.reshape(128, 2, 1)
             .astype(np.float32)).astype(NPFP8),
        bout=np.full((128, 1), float(np.asarray(b_out).reshape(-1)[0]),
                     np.float32),
        idn2=idn2.astype(NPFP8),
        sidn8=(np.eye(128, dtype=np.float32) * 0.125).astype(NPFP8),
        sidn4=(np.eye(128, dtype=np.float32) * 0.25).astype(NPFP8),
    )
    meta = dict(NCHUNK=tuple(int(x) for x in NCHUNK), TC=TC,
                nch=tuple(int(x) for r in nch for x in r),
                calls=tuple(calls), TOT=TOT, NIC=NIC,
                NQs=tuple((((part_bounds[i + 1] - part_bounds[i]) + 1) // 2
                           + 3) // 4 for i in range(NB)))
    return per_core, consts, meta, (core_of, local_of)


def _build(cfg, meta):
    NT, NPC, NPAD, HID, NC, P, NB = (cfg.NT, cfg.NPC, cfg.NPAD, cfg.HID,
                                     cfg.NC, cfg.P, cfg.NB)
    NCHUNK = meta["NCHUNK"]
    TC, NIC, TOT = meta["TC"], meta["NIC"], meta["TOT"]
    calls = meta["calls"]
    nch = np.array(meta["nch"], np.int64).reshape(NT, NB)
    tile_off = np.concatenate([[0], np.cumsum(NCHUNK)])
    NBLK = len(cfg.BLK)
    NQs = meta["NQs"]
    part_bounds = [int(tile_off[bt]) for bt in cfg.BT]

    nc = bacc.Bacc("TRN2", target_bir_lowering=False, debug=False,
                   num_swdge_queues=NQUEUES)
    x4_d = [nc.dram_tensor(f"x4_{i}", [128, max(NQs[i], 1) * 128], F32R,
                           kind="ExternalInput") for i in range(NB)]
    v1_d = nc.dram_tensor("v1", [128, TC], F32, kind="ExternalInput")
    s8_d = nc.dram_tensor("s8", [128, TOT * 128], FP8, kind="ExternalInput")
    idx_d = nc.dram_tensor("idx", [128, NIC], I16, kind="ExternalInput")
    wiT2_d = nc.dram_tensor("wit2", [128, 2 * HID], F32R,
                            kind="ExternalInput")
    w1s_d = nc.dram_tensor("w1s", [128, 2 * HID], FP8, kind="ExternalInput")
    w2s_d = nc.dram_tensor("w2s", [128, 2 * HID], FP8, kind="ExternalInput")
    wos_d = nc.dram_tensor("wos", [128, 2], FP8, kind="ExternalInput")
    bout_d = nc.dram_tensor("bout", [128, 1], F32, kind="ExternalInput")
    idn2_d = nc.dram_tensor("idn2", [128, 2 * 128], FP8, kind="ExternalInput")
    sidn8_d = nc.dram_tensor("sidn8", [128, 128], FP8, kind="ExternalInput")
    sidn4_d = nc.dram_tensor("sidn4", [128, 128], FP8, kind="ExternalInput")
    g_d = nc.dram_tensor("g", [1, NBLK * 512], F32, kind="ExternalOutput")

    bounce = [nc.dram_tensor(f"bounce{i}", [cfg.BROWS[i], HID], FP8)
              for i in range(NB)]
    table = [nc.dram_tensor(f"table{i}", [NC * cfg.BROWS[i], HID], FP8,
                            addr_space="Shared") for i in range(NB)]

    with tile.TileContext(nc) as tc:
        with (
            tc.tile_pool(name="const", bufs=1) as constp,
            tc.tile_pool(name="xs", bufs=3) as xsp,
            tc.tile_pool(name="msgs", bufs=6) as msgp,
            tc.tile_pool(name="sd", bufs=8) as sdp,
            tc.tile_pool(name="stage", bufs=3) as stagep,
            tc.tile_pool(name="resid", bufs=1) as residp,
            tc.tile_pool(name="pair", bufs=24) as pairp,
            tc.tile_pool(name="ph", bufs=2, space="PSUM") as php,
            tc.tile_pool(name="pz", bufs=2, space="PSUM") as pzp,
            tc.tile_pool(name="pt", bufs=2, space="PSUM") as ptp,
        ):
            wiT2 = constp.tile([128, 2 * HID], F32R, tag="wiT2", name="wiT2")
            nc.sync.dma_start(wiT2[:], wiT2_d[:])
            w1s = constp.tile([128, 2, HID], FP8, tag="w1s", name="w1s")
            nc.sync.dma_start(w1s[:], w1s_d[:])
            w2s = constp.tile([128, 2, HID], FP8, tag="w2s", name="w2s")
            nc.sync.dma_start(w2s[:], w2s_d[:])
            wos = constp.tile([128, 2, 1], FP8, tag="wos", name="wos")
            nc.sync.dma_start(wos[:], wos_d[:])
            bout = constp.tile([128, 1], F32, tag="bout", name="bout")
            nc.sync.dma_start(bout[:], bout_d[:])
            idn2 = constp.tile([128, 2, 128], FP8, tag="idn2", name="idn2")
            nc.sync.dma_start(idn2[:], idn2_d[:])
            sidn8 = constp.tile([128, 128], FP8, tag="sidn8", name="sidn8")
            nc.sync.dma_start(sidn8[:], sidn8_d[:])
            sidn4 = constp.tile([128, 128], FP8, tag="sidn4", name="sidn4")
            nc.sync.dma_start(sidn4[:], sidn4_d[:])
            v1 = constp.tile([128, TC], F32, tag="v1", name="v1")
            nc.sync.dma_start(v1[:], v1_d[:])
            idx = constp.tile([128, NIC], I16, tag="idx", name="idx")
            nc.sync.dma_start(idx[:], idx_d[:])

            ahT = residp.tile([128, 2, NPAD], FP8, tag="ahT", name="ahT")
            a2T = residp.tile([128, 2, NPAD], FP8, tag="a2T", name="a2T")
            partial = residp.tile([128, NT, HID], BF16, tag="part",
                                  name="part")

            # ---- phase A: hop 1 ------------------------------------------
            def epilogue_a(t, pz, used_right, part_i):
                ahb = stagep.tile([128, HID], FP8, tag="ahb", name="ahb")
                if used_right:
                    rh = stagep.tile([128, HID], BF16, tag="rh", name="rh")
                    nc.scalar.activation(rh[:], pz[:, HID:2 * HID], AF.Copy)
                    nc.vector.tensor_tensor(
                        ahb[:], pz[:, :HID], rh[:],
                        op=mybir.AluOpType.add)
                else:
                    nc.scalar.activation(ahb[:], pz[:, :HID], AF.Copy)
                r0 = t * 128 - cfg.BT[part_i] * 128
                rows = min(128, NPC - t * 128)
                nc.sync.dma_start(bounce[part_i][r0:r0 + rows, :],
                                  ahb[:rows, :])
                for mh in (0, 1):
                    pt = ptp.tile([128, 512], F32, tag="pt", name="pt")
                    nc.tensor.matmul(
                        pt[:, :128], lhsT=ahb[:, mh * 128:(mh + 1) * 128],
                        rhs=sidn8[:], start=True, stop=True,
                        skip_group_check=True)
                    nc.vector.tensor_copy(
                        ahT[:, mh, t * 128:(t + 1) * 128], pt[:, :128])

            def phase_a(part_i):
                lo, hi = part_bounds[part_i], part_bounds[part_i + 1]
                xd = x4_d[part_i]
                NQp = NQs[part_i]
                t = int(np.searchsorted(tile_off, lo, side="right")) - 1
                pz = None
                mq = None          # quad message tile [128, 2, 512]
                mq2 = None         # leftover pair tile [128, 2, 256]
                xs = None
                for p in range((hi - lo + 1) // 2):
                    if p % 4 == 0:
                        xs = xsp.tile([128, 128], F32R, tag="xs", name="xs")
                        q = p // 4
                        nc.sync.dma_start(xs[:],
                                          xd[:, q * 128:(q + 1) * 128])
                    j = p % 4
                    kA = lo + 2 * p
                    single = kA + 1 >= hi
                    ph = php.tile([128, 512], F32, tag="ph", name="ph",
                                  bufs=4)
                    nc.tensor.matmul(
                        ph[:, :2 * HID],
                        lhsT=xs[32 * j:32 * j + 8, :],
                        rhs=wiT2[32 * j:32 * j + 8, :],
                        start=True, stop=True, skip_group_check=True,
                        tile_position=(32 * j, 0))
                    for k in (kA,) if single else (kA, kA + 1):
                        if k == int(tile_off[t]):
                            pz = pzp.tile([128, 512], F32, tag="acc",
                                          name="acc")
                        nchk = int(NCHUNK[t])
                        q_in = k - int(tile_off[t])
                        nq = nchk // 4
                        tstart = q_in == 0
                        tlast = q_in == nchk - 1
                        ph_half = ph[:, (k - kA) * HID:(k - kA + 1) * HID]
                        # destination quarter for this chunk's message
                        if q_in < 4 * nq:
                            qq = q_in % 4
                            if qq == 0:
                                mq = msgp.tile([128, 2, 2 * HID], FP8,
                                               tag="mq", name="mq")
                            dst = mq[:, qq % 2, (qq // 2) * HID:
                                     (qq // 2 + 1) * HID]
                        else:
                            rr = q_in - 4 * nq
                            if rr == 0:
                                mq2 = msgp.tile([128, 2, HID], FP8,
                                                tag="mq2", name="mq2")
                            dst = mq2[:, rr % 2, :] if rr < 2 \
                                else mq2[:, 0, :]
                            if rr == 2:
                                mq2 = msgp.tile([128, 2, HID], FP8,
                                                tag="mq2", name="mq2")
                                dst = mq2[:, 0, :]
                        if k % 2 == 0:
                            nc.scalar.activation(dst, ph_half, AF.Relu,
                                                 scale=v1[:, k:k + 1])
                        else:
                            nc.vector.tensor_scalar(
                                dst, ph_half, v1[:, k:k + 1], 0.0,
                                op0=mybir.AluOpType.mult,
                                op1=mybir.AluOpType.max)
                        # emit accumulation matmuls
                        if q_in < 4 * nq and q_in % 4 == 3:
                            nc.tensor.matmul(
                                pz[:, :2 * HID], lhsT=idn2[:], rhs=mq[:],
                                perf_mode=DR, start=(q_in == 3),
                                stop=tlast, skip_group_check=True)
                        elif q_in >= 4 * nq:
                            rr = q_in - 4 * nq
                            rem = nchk - 4 * nq
                            if rr == 1 and rem >= 2:
                                nc.tensor.matmul(
                                    pz[:, :HID], lhsT=idn2[:], rhs=mq2[:],
                                    perf_mode=DR, start=(nq == 0 and rr == 1),
                                    stop=(q_in == nchk - 1),
                                    skip_group_check=True)
                            elif rr == 0 and rem == 1 or rr == 2:
                                nc.tensor.matmul(
                                    pz[:, :HID], lhsT=idn2[:, 0, :],
                                    rhs=mq2[:, 0, :],
                                    start=(nq == 0 and rr == 0),
                                    stop=tlast, skip_group_check=True)
                        if tlast:
                            epilogue_a(t, pz, nq > 0, part_i)
                            t += 1

            # ---- emit phase A parts + collectives + phase C passes -------
            for i in range(NB):
                phase_a(i)

            def collective(i):
                nc.gpsimd.collective_compute(
                    "AllGather", mybir.AluOpType.bypass,
                    replica_groups=[list(range(NC))],
                    ins=[bounce[i].ap().opt()],
                    outs=[table[i].ap().opt()],
                )

            collective(0)

            def dense_block(bidx):
                off, w = cfg.BLK[bidx]
                ht = stagep.tile([128, 2, 512], FP8, tag="h2t", name="h2t")
                for mh in (0, 1):
                    pd = pzp.tile([128, 512], F32, tag="acc", name="acc")
                    nc.tensor.matmul(
                        pd[:, :w], lhsT=w1s[:, :, mh * 128:(mh + 1) * 128],
                        rhs=ahT[:, :, off:off + w], perf_mode=DR,
                        start=True, stop=False, skip_group_check=True)
                    nc.tensor.matmul(
                        pd[:, :w], lhsT=w2s[:, :, mh * 128:(mh + 1) * 128],
                        rhs=a2T[:, :, off:off + w], perf_mode=DR,
                        start=False, stop=True, skip_group_check=True)
                    nc.scalar.activation(ht[:, mh, :w], pd[:, :w], AF.Relu,
                                         scale=0.015625)
                pg = ptp.tile([1, 512], F32, tag="pt", name="pg")
                for i in (0, 1):
                    nc.tensor.matmul(pg[:, :w], lhsT=wos[:, i, :],
                                     rhs=ht[:, i, :w],
                                     start=(i == 0), stop=(i == 1),
                                     skip_group_check=True)
                gb = stagep.tile([1, 512], F32, tag="gbuf", name="gb",
                                 bufs=4)
                nc.vector.tensor_copy(gb[0:1, :w], pg[:, :w])
                ge = stagep.tile([1, 512], F32, tag="gbuf", name="ge",
                                 bufs=4)
                nc.scalar.activation(ge[0:1, :w], gb[0:1, :w], AF.Exp,
                                     bias=bout[0:1, :], scale=0.0625)
                go = stagep.tile([1, 512], F32, tag="gbuf", name="go",
                                 bufs=4)
                nc.scalar.activation(go[0:1, :w], ge[0:1, :w], AF.Ln,
                                     bias=1.0)
                nc.sync.dma_start(g_d[0:1, off:off + w], go[0:1, :w])

            # phase C: one pass per bucket
            ci = 0
            col0 = 0
            qrr = 0
            for b in range(NB):
                bcalls = [cl for cl in calls if cl[1] == b]
                ncoll = max(1, int(COLL_FRAC * len(bcalls)))
                nc_done = 0
                for t in range(NT):
                    ncht = int(nch[t, b])
                    pz = pzp.tile([128, 512], F32, tag="acc", name="acc")
                    done = 0
                    while done < ncht:
                        (tt, bb, base, g) = calls[ci]
                        assert tt == t and bb == b
                        ni = g * 128
                        pr = pairp.tile([128, MAX_CALL_CHUNKS, HID], FP8,
                                        tag="pair", name="pair")
                        nc.gpsimd.dma_gather(
                            pr[:, :g, :], table[b].ap(),
                            idx[:, col0:col0 + ni // 16],
                            ni, ni, HID, single_packet=False,
                            queue_num=qrr)
                        qrr = (qrr + 1) % NQUEUES
                        sd = sdp.tile([128, MAX_CALL_CHUNKS, 128], FP8,
                                      tag="sdl", name="sdl")
                        nc.scalar.dma_start(
                            sd[:, :g, :],
                            s8_d[:, base * 128:(base + g) * 128])
                        for cc in range(0, g - 1, 2):
                            nc.tensor.matmul(
                                pz[:, :HID], lhsT=sd[:, cc:cc + 2, :],
                                rhs=pr[:, cc:cc + 2, :], perf_mode=DR,
                                start=(done + cc == 0),
                                stop=(done + cc + 2 == ncht),
                                skip_group_check=True)
                        if g % 2:
                            nc.tensor.matmul(
                                pz[:, :HID], lhsT=sd[:, g - 1, :],
                                rhs=pr[:, g - 1, :],
                                start=(done + g - 1 == 0),
                                stop=(done + g == ncht),
                                skip_group_check=True)
                        done += g
                        col0 += ni // 16
                        ci += 1
                        nc_done += 1
                        if b + 1 < NB and nc_done == ncoll:
                            collective(b + 1)
                    # combine into partial / final epilogue
                    if b + 1 < NB:
                        if b == 0:
                            nc.vector.tensor_scalar(
                                partial[:, t, :], pz[:, :HID], 0.0625, 0.0,
                                op0=mybir.AluOpType.mult,
                                op1=mybir.AluOpType.bypass)
                        else:
                            t1 = stagep.tile([128, HID], BF16, tag="t1",
                                             name="t1")
                            nc.vector.tensor_scalar(
                                t1[:], pz[:, :HID], 0.0625, 0.0,
                                op0=mybir.AluOpType.mult,
                                op1=mybir.AluOpType.bypass)
                            nc.vector.tensor_tensor(
                                partial[:, t, :], partial[:, t, :], t1[:],
                                op=mybir.AluOpType.add)
                    else:
                        t1 = stagep.tile([128, HID], BF16, tag="t1",
                                         name="t1")
                        nc.vector.tensor_scalar(
                            t1[:], pz[:, :HID], 0.0625, 0.0,
                            op0=mybir.AluOpType.mult,
                            op1=mybir.AluOpType.bypass)
                        a2b = stagep.tile([128, HID], FP8, tag="a2b",
                                          name="a2b")
                        nc.vector.tensor_tensor(
                            a2b[:], partial[:, t, :], t1[:],
                            op=mybir.AluOpType.add)
                        for mh in (0, 1):
                            pt = ptp.tile([128, 512], F32, tag="pt",
                                          name="pt")
                            nc.tensor.matmul(
                                pt[:, :128],
                                lhsT=a2b[:, mh * 128:(mh + 1) * 128],
                                rhs=sidn4[:], start=True, stop=True,
                                skip_group_check=True)
                            nc.vector.tensor_copy(
                                a2T[:, mh, t * 128:(t + 1) * 128],
                                pt[:, :128])
                        if t % 4 == 3:
                            dense_block(t // 4)
            for bidx in range(NT // 4, NBLK):
                dense_block(bidx)

    nc.compile()
    return nc


_COMPILED = {}


def _get_compiled(cfg, meta):
    key = (cfg.P, cfg.E, meta["NCHUNK"], meta["nch"], meta["calls"])
    if key not in _COMPILED:
        _COMPILED[key] = _build(cfg, meta)
    return _COMPILED[key]


def run(cfg, inputs, trace=False):
    per_core, consts, meta, (core_of, local_of) = _prepare(cfg, **inputs)
    ncobj = _get_compiled(cfg, meta)
    in_maps = []
    for c in range(cfg.NC):
        im = dict(per_core[c])
        im.update({k: np.asarray(v) for k, v in consts.items()})
        in_maps.append(im)
    res = run_bass_kernel_spmd(ncobj, in_maps, list(range(cfg.NC)),
                               trace=trace)
    g = np.empty(cfg.P, np.float32)
    for c in range(cfg.NC):
        go = np.asarray(res.results[c]["g"]).reshape(-1)
        mine = core_of == c
        g[mine] = go[local_of[mine]]
    return g.reshape(cfg.P, 1), res


def kernel(**inputs):
    cfg = Cfg(P=50000, E=800000)
    g, _ = run(cfg, inputs)
    return g


# revision 21
# speedup vs baseline: 1.0300x; 1.0300x over previous
"""BetaGNN message-passing kernel for 8 Trainium2 NeuronCores.

Strategy (dest-row sharding, 6250 nodes/core):
  - Host relabels nodes: sorted by in-degree, dealt round-robin to cores so
    every core's tile t has near-identical max-degree -> uniform chunk counts.
  - Hop 1 (AH = A @ relu(x @ W_in^T + b)): no gather. Host pre-gathers the
    3-wide input features per edge (plus a ones column); the PE recomputes h
    per edge-slot, TWO chunks per matmul (K=8 block-diagonal W_in, N=512).
    Edge values (x16) fold into the relu via per-partition scale; fp8
    messages accumulate FOUR chunks per DoubleRow identity matmul into a
    split [128,512] accumulator whose halves are summed in the epilogue.
  - Local AH rows (x16, fp8) are AllGathered in THREE slices, each fired as
    soon as its tiles finish so collectives overlap hop-1 compute and the
    early hop-2 gathers. Each slice lands in a compact table so gather
    indices stay int16.
  - Hop 2 (A2H = A @ AH): edges are bucketed by source slice and packed
    128/chunk with a general scatter matrix S (fp8, x16) routing
    slot -> dest row. Rows are dma_gathered (256B fp8) on 4 SWDGE queues;
    pairs of chunks accumulate with one DoubleRow matmul. Buckets are
    processed in separate passes (bf16 partials staged in SBUF) so a
    not-yet-ready collective never head-of-line blocks the gather queue;
    the next collective's dispatch is emitted in the middle of the previous
    bucket's gather stream.
  - Dense tail in transposed layout: AH/A2H tiles transpose via fp8 matmuls
    against scaled identities into [128, 2, NPAD] fp8 residents;
    h2^T = relu(W1 AH^T + W2 A2H^T) (DoubleRow over the two hid halves) and
    g = softplus(W_out h2^T + b_out), one 512-col block at a time,
    interleaved into the last hop-2 pass. All fp8 scale factors are powers
    of two (exact).
"""

import sys

for _p in ("/opt/trn_rl_repo", "/root/.axon_site/_ro/trn_rl_repo"):
    if _p not in sys.path:
        sys.path.insert(0, _p)

import numpy as np
import ml_dtypes

import concourse.bacc as bacc
import concourse.bass as bass
import concourse.mybir as mybir
from concourse import tile
from concourse.bass_utils import run_bass_kernel_spmd

F32 = mybir.dt.float32
F32R = mybir.dt.float32r
BF16 = mybir.dt.bfloat16
FP8 = mybir.dt.float8e4
I16 = mybir.dt.int16
AF = mybir.ActivationFunctionType
DR = mybir.MatmulPerfMode.DoubleRow
NPFP8 = ml_dtypes.float8_e4m3fn

MAX_CALL_CHUNKS = 12      # <=12 chunks (1536 idxs) per dma_gather call
NQUEUES = 4               # SWDGE queues for gather concurrency
COLL_FRAC = 0.6           # emit next collective after this fraction of calls


class Cfg:
    def __init__(self, P, E, nc=8, hid=256):
        assert P % (nc * 2) == 0
        self.P, self.E, self.NC, self.HID = P, E, nc, hid
        self.NPC = P // nc                    # nodes per core
        self.NT = (self.NPC + 127) // 128     # dest tiles per core
        self.NPAD = self.NT * 128
        if self.NT > 40:
            self.SPLITS = [17, 34]            # bucket boundaries (tiles)
        else:
            self.SPLITS = [max(1, self.NT // 2)]
        bounds = [0] + self.SPLITS + [self.NT]
        self.NB = len(bounds) - 1
        self.BROWS = []                       # locals per bucket
        for i in range(self.NB):
            lo = bounds[i] * 128
            hi = min(bounds[i + 1] * 128, self.NPC)
            self.BROWS.append(hi - lo)
        self.BT = bounds                      # tile bounds per bucket
        self.BLK = []
        off = 0
        while off < self.NPAD:
            w = min(512, self.NPAD - off)
            self.BLK.append((off, w))
            off += w


def _plan(cfg, deg):
    """Hop-1 plan: chunk count per tile = max in-degree in the tile."""
    P, NC, NT = cfg.P, cfg.NC, cfg.NT
    order = np.argsort(-deg, kind="stable")
    rank = np.empty(P, np.int64)
    rank[order] = np.arange(P)
    core_of = rank % NC
    local_of = rank // NC
    gid = core_of * cfg.NPC + local_of
    degs_sorted = deg[order]
    NCHUNK = []
    for t in range(NT):
        NCHUNK.append(max(1, int(degs_sorted[min(t * 128 * NC, P - 1)])))
    NCHUNK = np.array(NCHUNK, np.int64)
    tile_off = np.concatenate([[0], np.cumsum(NCHUNK)])
    return core_of, local_of, gid, NCHUNK, tile_off, int(tile_off[-1])


def _split_calls(nchunks):
    """Split a chunk count into gather calls <= MAX_CALL_CHUNKS, keeping
    every non-final call even so DoubleRow pairs never straddle calls."""
    out = []
    rem = nchunks
    while rem:
        g = min(MAX_CALL_CHUNKS, rem)
        if g < rem and g % 2:
            g -= 1
        out.append(g)
        rem -= g
    return out


def _pack_pairs(x4T, lo, hi):
    """Pack chunks [lo,hi) of x4T ([4, TC*128]) in h-pair layout: pair p ->
    partitions 32*(p%4)+(0..8), col block p//4. Odd tail chunk packs alone
    in the A-half of its pair slot."""
    n = hi - lo
    npr = (n + 1) // 2
    NQ = (npr + 3) // 4
    x4q = np.zeros((128, NQ * 128), np.float32)
    for p in range(npr):
        j, q = p % 4, p // 4
        kA = lo + 2 * p
        x4q[32 * j:32 * j + 4, q * 128:(q + 1) * 128] = \
            x4T[:, kA * 128:(kA + 1) * 128]
        if 2 * p + 1 < n:
            kB = kA + 1
            x4q[32 * j + 4:32 * j + 8, q * 128:(q + 1) * 128] = \
                x4T[:, kB * 128:(kB + 1) * 128]
    return x4q, NQ


def _prepare(cfg, beta, degree, A_rows, A_cols, A_vals,
             W_in, b_in, W_mp1, W_mp2, W_out, b_out):
    P, E, NC, NPC, NT = cfg.P, cfg.E, cfg.NC, cfg.NPC, cfg.NT
    NB = cfg.NB
    deg = np.bincount(A_rows, minlength=P).astype(np.int64)
    core_of, local_of, gid, NCHUNK, tile_off, TC = _plan(cfg, deg)

    # ---- hop-1 edge slots (slot column == dest column) ----
    d_gid = gid[A_rows.astype(np.int64)]
    oe = np.argsort(d_gid, kind="stable")
    sd = d_gid[oe]
    first = np.r_[True, sd[1:] != sd[:-1]]
    cumstart = np.maximum.accumulate(np.where(first, np.arange(E), 0))
    chunk = np.arange(E) - cumstart
    e_core = sd // NPC
    e_local = sd % NPC
    e_col = e_local % 128
    e_k = tile_off[e_local // 128] + chunk
    e_slot = e_k * 128 + e_col
    src1 = A_cols.astype(np.int64)[oe]
    vals1 = A_vals[oe].astype(np.float32)

    x4_all = np.stack([beta[:, 0], beta[:, 0] ** 2, degree[:, 0],
                       np.ones(P, np.float32)], axis=0).astype(np.float32)

    # ---- hop-2 edge plan: sort by (core, tile, bucket) ----
    s_gid = gid[A_cols.astype(np.int64)]
    c2_core = d_gid // NPC
    c2_loc = d_gid % NPC
    c2_tile = c2_loc // 128
    c2_dcol = c2_loc % 128
    s_loc = s_gid % NPC
    s_core = s_gid // NPC
    blo = np.array([cfg.BT[i] * 128 for i in range(NB)], np.int64)
    c2_b = np.searchsorted(blo, s_loc, side="right") - 1
    brows = np.array(cfg.BROWS, np.int64)
    c2_tidx = s_core * brows[c2_b] + (s_loc - blo[c2_b])
    o2 = np.lexsort((c2_b, c2_tile, c2_core))
    g_core = c2_core[o2]
    g_tile = c2_tile[o2]
    g_b = c2_b[o2]
    g_dcol = c2_dcol[o2]
    g_tidx = c2_tidx[o2]
    g_val = A_vals[o2].astype(np.float32)
    key = (g_core * NT + g_tile) * NB + g_b
    kfirst = np.r_[True, key[1:] != key[:-1]]
    kcum = np.maximum.accumulate(np.where(kfirst, np.arange(E), 0))
    g_pos = np.arange(E) - kcum

    # shared SPMD structure (max over cores, min 1 chunk per (t,b))
    cnt_all = np.zeros((NC, NT, NB), np.int64)
    np.add.at(cnt_all, (g_core, g_tile, g_b), 1)
    nch = np.maximum(1, -(-cnt_all.max(axis=0) // 128))   # [NT, NB]
    flat = nch.reshape(-1)
    cbase = np.concatenate([[0], np.cumsum(flat)]).astype(np.int64)
    TOT = int(cbase[-1])
    # calls grouped bucket-major (pass order)
    calls = []          # (tile, bucket, chunk_base, g)
    for b in range(NB):
        for t in range(NT):
            base = int(cbase[t * NB + b])
            for g in _split_calls(int(nch[t, b])):
                calls.append((t, b, base, g))
                base += g
    NIC = sum(g * 128 // 16 for (_, _, _, g) in calls)

    part_bounds = [int(tile_off[bt]) for bt in cfg.BT]    # chunk bounds

    per_core = []
    for c in range(NC):
        # ---- hop 1 arrays ----
        m1 = e_core == c
        sl1 = e_slot[m1]
        x4T = np.zeros((4, TC * 128), np.float32)
        x4T[:, sl1] = x4_all[:, src1[m1]]
        v1 = np.zeros((128, TC), np.float32)
        v1[e_col[m1], e_k[m1]] = 16.0 * vals1[m1]
        xparts = []
        for i in range(NB):
            x4q, NQ = _pack_pairs(x4T, part_bounds[i], part_bounds[i + 1])
            xparts.append(x4q)

        # ---- hop 2 arrays ----
        m2 = g_core == c
        e_key = (g_tile[m2] * NB + g_b[m2])
        e_chunk = cbase[e_key] + g_pos[m2] // 128
        e_p = g_pos[m2] % 128
        S8 = np.zeros((128, TOT * 128), np.float32)
        S8[e_p, e_chunk * 128 + g_dcol[m2]] = 16.0 * g_val[m2]
        S8 = S8.astype(NPFP8)
        slot_idx = np.zeros(TOT * 128, np.int64)
        slot_idx[e_chunk * 128 + e_p] = g_tidx[m2]

        idxh = np.zeros((128, NIC), np.int16)
        col0 = 0
        for (t, b, base, g) in calls:
            ni = g * 128
            blockv = slot_idx[base * 128:base * 128 + ni].astype(np.int16)
            blockv = blockv.reshape(ni // 16, 16).T
            for q in range(8):
                idxh[16 * q:16 * (q + 1), col0:col0 + ni // 16] = blockv
            col0 += ni // 16
        pc = dict(v1=v1, s8=S8, idx=idxh)
        for i in range(NB):
            pc[f"x4_{i}"] = xparts[i]
        per_core.append(pc)

    # ---- constants (power-of-two scaled for fp8) ----
    wiT = np.concatenate([W_in.T.astype(np.float32),
                          b_in[None, :].astype(np.float32)], axis=0)
    HID = cfg.HID
    wiT2 = np.zeros((128, 2 * HID), np.float32)
    for j in range(4):
        wiT2[32 * j:32 * j + 4, 0:HID] = wiT
        wiT2[32 * j + 4:32 * j + 8, HID:2 * HID] = wiT

    def pack_w(W, scale):
        w = (scale * W.T.astype(np.float32)).reshape(2, 128, W.shape[0])
        return np.ascontiguousarray(w.transpose(1, 0, 2)).astype(NPFP8)

    idn2 = np.zeros((128, 2, 128), np.float32)
    idn2[np.arange(128), 0, np.arange(128)] = 1.0
    idn2[np.arange(128), 1, np.arange(128)] = 1.0

    consts = dict(
        wit2=wiT2,
        w1s=pack_w(W_mp1, 32.0),
        w2s=pack_w(W_mp2, 16.0),
        wos=(16.0 * W_out.reshape(2, 128).T.reshape(128, 2, 1)
             .astype(np.float32)).astype(NPFP8),
        bout=np.full((128, 1), float(np.asarray(b_out).reshape(-1)[0]),
                     np.float32),
        idn2=idn2.astype(NPFP8),
        sidn8=(np.eye(128, dtype=np.float32) * 0.125).astype(NPFP8),
        sidn4=(np.eye(128, dtype=np.float32) * 0.25).astype(NPFP8),
    )
    meta = dict(NCHUNK=tuple(int(x) for x in NCHUNK), TC=TC,
                nch=tuple(int(x) for r in nch for x in r),
                calls=tuple(calls), TOT=TOT, NIC=NIC,
                NQs=tuple((((part_bounds[i + 1] - part_bounds[i]) + 1) // 2
                           + 3) // 4 for i in range(NB)))
    return per_core, consts, meta, (core_of, local_of)


def _build(cfg, meta):
    NT, NPC, NPAD, HID, NC, P, NB = (cfg.NT, cfg.NPC, cfg.NPAD, cfg.HID,
                                     cfg.NC, cfg.P, cfg.NB)
    NCHUNK = meta["NCHUNK"]
    TC, NIC, TOT = meta["TC"], meta["NIC"], meta["TOT"]
    calls = meta["calls"]
    nch = np.array(meta["nch"], np.int64).reshape(NT, NB)
    tile_off = np.concatenate([[0], np.cumsum(NCHUNK)])
    NBLK = len(cfg.BLK)
    NQs = meta["NQs"]
    part_bounds = [int(tile_off[bt]) for bt in cfg.BT]

    nc = bacc.Bacc("TRN2", target_bir_lowering=False, debug=False,
                   num_swdge_queues=NQUEUES)
    x4_d = [nc.dram_tensor(f"x4_{i}", [128, max(NQs[i], 1) * 128], F32R,
                           kind="ExternalInput") for i in range(NB)]
    v1_d = nc.dram_tensor("v1", [128, TC], F32, kind="ExternalInput")
    s8_d = nc.dram_tensor("s8", [128, TOT * 128], FP8, kind="ExternalInput")
    idx_d = nc.dram_tensor("idx", [128, NIC], I16, kind="ExternalInput")
    wiT2_d = nc.dram_tensor("wit2", [128, 2 * HID], F32R,
                            kind="ExternalInput")
    w1s_d = nc.dram_tensor("w1s", [128, 2 * HID], FP8, kind="ExternalInput")
    w2s_d = nc.dram_tensor("w2s", [128, 2 * HID], FP8, kind="ExternalInput")
    wos_d = nc.dram_tensor("wos", [128, 2], FP8, kind="ExternalInput")
    bout_d = nc.dram_tensor("bout", [128, 1], F32, kind="ExternalInput")
    idn2_d = nc.dram_tensor("idn2", [128, 2 * 128], FP8, kind="ExternalInput")
    sidn8_d = nc.dram_tensor("sidn8", [128, 128], FP8, kind="ExternalInput")
    sidn4_d = nc.dram_tensor("sidn4", [128, 128], FP8, kind="ExternalInput")
    g_d = nc.dram_tensor("g", [1, NBLK * 512], F32, kind="ExternalOutput")

    bounce = [nc.dram_tensor(f"bounce{i}", [cfg.BROWS[i], HID], FP8)
              for i in range(NB)]
    table = [nc.dram_tensor(f"table{i}", [NC * cfg.BROWS[i], HID], FP8,
                            addr_space="Shared") for i in range(NB)]

    with tile.TileContext(nc) as tc:
        with (
            tc.tile_pool(name="const", bufs=1) as constp,
            tc.tile_pool(name="xs", bufs=3) as xsp,
            tc.tile_pool(name="msgs", bufs=6) as msgp,
            tc.tile_pool(name="sd", bufs=8) as sdp,
            tc.tile_pool(name="stage", bufs=3) as stagep,
            tc.tile_pool(name="resid", bufs=1) as residp,
            tc.tile_pool(name="pair", bufs=24) as pairp,
            tc.tile_pool(name="ph", bufs=2, space="PSUM") as php,
            tc.tile_pool(name="pz", bufs=2, space="PSUM") as pzp,
            tc.tile_pool(name="pt", bufs=2, space="PSUM") as ptp,
        ):
            wiT2 = constp.tile([128, 2 * HID], F32R, tag="wiT2", name="wiT2")
            nc.sync.dma_start(wiT2[:], wiT2_d[:])
            w1s = constp.tile([128, 2, HID], FP8, tag="w1s", name="w1s")
            nc.sync.dma_start(w1s[:], w1s_d[:])
            w2s = constp.tile([128, 2, HID], FP8, tag="w2s", name="w2s")
            nc.sync.dma_start(w2s[:], w2s_d[:])
            wos = constp.tile([128, 2, 1], FP8, tag="wos", name="wos")
            nc.sync.dma_start(wos[:], wos_d[:])
            bout = constp.tile([128, 1], F32, tag="bout", name="bout")
            nc.sync.dma_start(bout[:], bout_d[:])
            idn2 = constp.tile([128, 2, 128], FP8, tag="idn2", name="idn2")
            nc.sync.dma_start(idn2[:], idn2_d[:])
            sidn8 = constp.tile([128, 128], FP8, tag="sidn8", name="sidn8")
            nc.sync.dma_start(sidn8[:], sidn8_d[:])
            sidn4 = constp.tile([128, 128], FP8, tag="sidn4", name="sidn4")
            nc.sync.dma_start(sidn4[:], sidn4_d[:])
            v1 = constp.tile([128, TC], F32, tag="v1", name="v1")
            nc.sync.dma_start(v1[:], v1_d[:])
            idx = constp.tile([128, NIC], I16, tag="idx", name="idx")
            nc.sync.dma_start(idx[:], idx_d[:])

            ahT = residp.tile([128, 2, NPAD], FP8, tag="ahT", name="ahT")
            a2T = residp.tile([128, 2, NPAD], FP8, tag="a2T", name="a2T")
            partial = residp.tile([128, NT, HID], BF16, tag="part",
                                  name="part")

            # ---- phase A: hop 1 ------------------------------------------
            def epilogue_a(t, pz, used_right, part_i):
                ahb = stagep.tile([128, HID], FP8, tag="ahb", name="ahb")
                if used_right:
                    rh = stagep.tile([128, HID], BF16, tag="rh", name="rh")
                    nc.scalar.activation(rh[:], pz[:, HID:2 * HID], AF.Copy)
                    nc.vector.tensor_tensor(
                        ahb[:], pz[:, :HID], rh[:],
                        op=mybir.AluOpType.add)
                else:
                    nc.scalar.activation(ahb[:], pz[:, :HID], AF.Copy)
                r0 = t * 128 - cfg.BT[part_i] * 128
                rows = min(128, NPC - t * 128)
                nc.sync.dma_start(bounce[part_i][r0:r0 + rows, :],
                                  ahb[:rows, :])
                for mh in (0, 1):
                    pt = ptp.tile([128, 512], F32, tag="pt", name="pt")
                    nc.tensor.matmul(
                        pt[:, :128], lhsT=ahb[:, mh * 128:(mh + 1) * 128],
                        rhs=sidn8[:], start=True, stop=True,
                        skip_group_check=True)
                    nc.vector.tensor_copy(
                        ahT[:, mh, t * 128:(t + 1) * 128], pt[:, :128])

            def phase_a(part_i):
                lo, hi = part_bounds[part_i], part_bounds[part_i + 1]
                xd = x4_d[part_i]
                NQp = NQs[part_i]
                t = int(np.searchsorted(tile_off, lo, side="right")) - 1
                pz = None
                mq = None          # quad message tile [128, 2, 512]
                mq2 = None         # leftover pair tile [128, 2, 256]
                xs = None
                for p in range((hi - lo + 1) // 2):
                    if p % 4 == 0:
                        xs = xsp.tile([128, 128], F32R, tag="xs", name="xs")
                        q = p // 4
                        nc.sync.dma_start(xs[:],
                                          xd[:, q * 128:(q + 1) * 128])
                    j = p % 4
                    kA = lo + 2 * p
                    single = kA + 1 >= hi
                    ph = php.tile([128, 512], F32, tag="ph", name="ph",
                                  bufs=4)
                    nc.tensor.matmul(
                        ph[:, :2 * HID],
                        lhsT=xs[32 * j:32 * j + 8, :],
                        rhs=wiT2[32 * j:32 * j + 8, :],
                        start=True, stop=True, skip_group_check=True,
                        tile_position=(32 * j, 0))
                    for k in (kA,) if single else (kA, kA + 1):
                        if k == int(tile_off[t]):
                            pz = pzp.tile([128, 512], F32, tag="acc",
                                          name="acc")
                        nchk = int(NCHUNK[t])
                        q_in = k - int(tile_off[t])
                        nq = nchk // 4
                        tstart = q_in == 0
                        tlast = q_in == nchk - 1
                        ph_half = ph[:, (k - kA) * HID:(k - kA + 1) * HID]
                        # destination quarter for this chunk's message
                        if q_in < 4 * nq:
                            qq = q_in % 4
                            if qq == 0:
                                mq = msgp.tile([128, 2, 2 * HID], FP8,
                                               tag="mq", name="mq")
                            dst = mq[:, qq % 2, (qq // 2) * HID:
                                     (qq // 2 + 1) * HID]
                        else:
                            rr = q_in - 4 * nq
                            if rr == 0:
                                mq2 = msgp.tile([128, 2, HID], FP8,
                                                tag="mq2", name="mq2")
                            dst = mq2[:, rr % 2, :] if rr < 2 \
                                else mq2[:, 0, :]
                            if rr == 2:
                                mq2 = msgp.tile([128, 2, HID], FP8,
                                                tag="mq2", name="mq2")
                                dst = mq2[:, 0, :]
                        if k % 2 == 0:
                            nc.scalar.activation(dst, ph_half, AF.Relu,
                                                 scale=v1[:, k:k + 1])
                        else:
                            nc.vector.tensor_scalar(
                                dst, ph_half, v1[:, k:k + 1], 0.0,
                                op0=mybir.AluOpType.mult,
                                op1=mybir.AluOpType.max)
                        # emit accumulation matmuls
                        if q_in < 4 * nq and q_in % 4 == 3:
                            nc.tensor.matmul(
                                pz[:, :2 * HID], lhsT=idn2[:], rhs=mq[:],
                                perf_mode=DR, start=(q_in == 3),
                                stop=tlast, skip_group_check=True)
                        elif q_in >= 4 * nq:
                            rr = q_in - 4 * nq
                            rem = nchk - 4 * nq
                            if rr == 1 and rem >= 2:
                                nc.tensor.matmul(
                                    pz[:, :HID], lhsT=idn2[:], rhs=mq2[:],
                                    perf_mode=DR, start=(nq == 0 and rr == 1),
                                    stop=(q_in == nchk - 1),
                                    skip_group_check=True)
                            elif rr == 0 and rem == 1 or rr == 2:
                                nc.tensor.matmul(
                                    pz[:, :HID], lhsT=idn2[:, 0, :],
                                    rhs=mq2[:, 0, :],
                                    start=(nq == 0 and rr == 0),
                                    stop=tlast, skip_group_check=True)
                        if tlast:
                            epilogue_a(t, pz, nq > 0, part_i)
                            t += 1

            # ---- emit phase A parts + collectives + phase C passes -------
            for i in range(NB):
                phase_a(i)

            def collective(i):
                nc.gpsimd.collective_compute(
                    "AllGather", mybir.AluOpType.bypass,
                    replica_groups=[list(range(NC))],
                    ins=[bounce[i].ap().opt()],
                    outs=[table[i].ap().opt()],
                )

            collective(0)

            def dense_block(bidx):
                off, w = cfg.BLK[bidx]
                ht = stagep.tile([128, 2, 512], FP8, tag="h2t", name="h2t")
                for mh in (0, 1):
                    pd = pzp.tile([128, 512], F32, tag="acc", name="acc")
                    nc.tensor.matmul(
                        pd[:, :w], lhsT=w1s[:, :, mh * 128:(mh + 1) * 128],
                        rhs=ahT[:, :, off:off + w], perf_mode=DR,
                        start=True, stop=False, skip_group_check=True)
                    nc.tensor.matmul(
                        pd[:, :w], lhsT=w2s[:, :, mh * 128:(mh + 1) * 128],
                        rhs=a2T[:, :, off:off + w], perf_mode=DR,
                        start=False, stop=True, skip_group_check=True)
                    nc.scalar.activation(ht[:, mh, :w], pd[:, :w], AF.Relu,
                                         scale=0.015625)
                pg = ptp.tile([1, 512], F32, tag="pt", name="pg")
                for i in (0, 1):
                    nc.tensor.matmul(pg[:, :w], lhsT=wos[:, i, :],
                                     rhs=ht[:, i, :w],
                                     start=(i == 0), stop=(i == 1),
                                     skip_group_check=True)
                gb = stagep.tile([1, 512], F32, tag="gbuf", name="gb",
                                 bufs=4)
                nc.vector.tensor_copy(gb[0:1, :w], pg[:, :w])
                ge = stagep.tile([1, 512], F32, tag="gbuf", name="ge",
                                 bufs=4)
                nc.scalar.activation(ge[0:1, :w], gb[0:1, :w], AF.Exp,
                                     bias=bout[0:1, :], scale=0.0625)
                go = stagep.tile([1, 512], F32, tag="gbuf", name="go",
                                 bufs=4)
                nc.scalar.activation(go[0:1, :w], ge[0:1, :w], AF.Ln,
                                     bias=1.0)
                nc.sync.dma_start(g_d[0:1, off:off + w], go[0:1, :w])

            # phase C: one pass per bucket
            ci = 0
            col0 = 0
            qrr = 0
            for b in range(NB):
                bcalls = [cl for cl in calls if cl[1] == b]
                ncoll = max(1, int(COLL_FRAC * len(bcalls)))
                nc_done = 0
                for t in range(NT):
                    ncht = int(nch[t, b])
                    pz = pzp.tile([128, 512], F32, tag="acc", name="acc")
                    done = 0
                    while done < ncht:
                        (tt, bb, base, g) = calls[ci]
                        assert tt == t and bb == b
                        ni = g * 128
                        pr = pairp.tile([128, MAX_CALL_CHUNKS, HID], FP8,
                                        tag="pair", name="pair")
                        nc.gpsimd.dma_gather(
                            pr[:, :g, :], table[b].ap(),
                            idx[:, col0:col0 + ni // 16],
                            ni, ni, HID, single_packet=False,
                            queue_num=qrr)
                        qrr = (qrr + 1) % NQUEUES
                        sd = sdp.tile([128, MAX_CALL_CHUNKS, 128], FP8,
                                      tag="sdl", name="sdl")
                        nc.scalar.dma_start(
                            sd[:, :g, :],
                            s8_d[:, base * 128:(base + g) * 128])
                        for cc in range(0, g - 1, 2):
                            nc.tensor.matmul(
                                pz[:, :HID], lhsT=sd[:, cc:cc + 2, :],
                                rhs=pr[:, cc:cc + 2, :], perf_mode=DR,
                                start=(done + cc == 0),
                                stop=(done + cc + 2 == ncht),
                                skip_group_check=True)
                        if g % 2:
                            nc.tensor.matmul(
                                pz[:, :HID], lhsT=sd[:, g - 1, :],
                                rhs=pr[:, g - 1, :],
                                start=(done + g - 1 == 0),
                                stop=(done + g == ncht),
                                skip_group_check=True)
                        done += g
                        col0 += ni // 16
                        ci += 1
                        nc_done += 1
                        if b + 1 < NB and nc_done == ncoll:
                            collective(b + 1)
                    # combine into partial / final epilogue
                    if b + 1 < NB:
                        if b == 0:
                            nc.vector.tensor_scalar(
                                partial[:, t, :], pz[:, :HID], 0.0625, 0.0,
                                op0=mybir.AluOpType.mult,
                                op1=mybir.AluOpType.bypass)
                        else:
                            t1 = stagep.tile([128, HID], BF16, tag="t1",
                                             name="t1")
                            nc.vector.tensor_scalar(
                                t1[:], pz[:, :HID], 0.0625, 0.0,
                                op0=mybir.AluOpType.mult,
                                op1=mybir.AluOpType.bypass)
                            nc.vector.tensor_tensor(
                                partial[:, t, :], partial[:, t, :], t1[:],
                                op=mybir.AluOpType.add)
                    else:
                        t1 = stagep.tile([128, HID], BF16, tag="t1",
                                         name="t1")
                        nc.vector.tensor_scalar(
                            t1[:], pz[:, :HID], 0.0625, 0.0,
                            op0=mybir.AluOpType.mult,
                            op1=mybir.AluOpType.bypass)
                        a2b = stagep.tile([128, HID], FP8, tag="a2b",
                                          name="a2b")
                        nc.vector.tensor_tensor(
                            a2b[:], partial[:, t, :], t1[:],
                            op=mybir.AluOpType.add)
                        for mh in (0, 1):
                            pt = ptp.tile([128, 512], F32, tag="pt",
                                          name="pt")
                            nc.tensor.matmul(
                                pt[:, :128],
                                lhsT=a2b[:, mh * 128:(mh + 1) * 128],
                                rhs=sidn4[:], start=True, stop=True,
                                skip_group_check=True)
                            nc.vector.tensor_copy(
                                a2T[:, mh, t * 128:(t + 1) * 128],
                                pt[:, :128])
                        if t % 4 == 3:
                            dense_block(t // 4)
            for bidx in range(NT // 4, NBLK):
                dense_block(bidx)

    nc.compile()
    return nc


_COMPILED = {}


def _get_compiled(cfg, meta):
    key = (cfg.P, cfg.E, meta["NCHUNK"], meta["nch"], meta["calls"])
    if key not in _COMPILED:
        _COMPILED[key] = _build(cfg, meta)
    return _COMPILED[key]


def run(cfg, inputs, trace=False):
    per_core, consts, meta, (core_of, local_of) = _prepare(cfg, **inputs)
    ncobj = _get_compiled(cfg, meta)
    in_maps = []
    for c in range(cfg.NC):
        im = dict(per_core[c])
        im.update({k: np.asarray(v) for k, v in consts.items()})
        in_maps.append(im)
    res = run_bass_kernel_spmd(ncobj, in_maps, list(range(cfg.NC)),
                               trace=trace)
    g = np.empty(cfg.P, np.float32)
    for c in range(cfg.NC):
        go = np.asarray(res.results[c]["g"]).reshape(-1)
        mine = core_of == c
        g[mine] = go[local_of[mine]]
    return g.reshape(cfg.P, 1), res


def kernel(**inputs):
    cfg = Cfg(P=50000, E=800000)
    g, _ = run(cfg, inputs)
    return g


# revision 22
# speedup vs baseline: 1.0536x; 1.0229x over previous
"""BetaGNN message-passing kernel for 8 Trainium2 NeuronCores.

Strategy (dest-row sharding, 6250 nodes/core):
  - Host relabels nodes: sorted by in-degree, dealt round-robin to cores so
    every core's tile t has near-identical max-degree -> uniform chunk counts.
  - Hop 1 (AH = A @ relu(x @ W_in^T + b)): no gather. Host pre-gathers the
    3-wide input features per edge (plus a ones column); the PE recomputes h
    per edge-slot, TWO chunks per matmul (K=8 block-diagonal W_in, N=512).
    Edge values (x16) fold into the relu via per-partition scale; fp8
    messages accumulate FOUR chunks per DoubleRow identity matmul into a
    split [128,512] accumulator whose halves are summed in the epilogue.
  - Local AH rows (x16, fp8) are AllGathered in THREE slices, each fired as
    soon as its tiles finish so collectives overlap hop-1 compute and the
    early hop-2 gathers. Each slice lands in a compact table so gather
    indices stay int16.
  - Hop 2 (A2H = A @ AH): edges are bucketed by source slice and packed
    128/chunk with a general scatter matrix S (fp8, x16) routing
    slot -> dest row. Rows are dma_gathered (256B fp8) on 4 SWDGE queues;
    pairs of chunks accumulate with one DoubleRow matmul. Buckets are
    processed in separate passes (bf16 partials staged in SBUF) so a
    not-yet-ready collective never head-of-line blocks the gather queue;
    the next collective's dispatch is emitted in the middle of the previous
    bucket's gather stream.
  - Dense tail in transposed layout: AH/A2H tiles transpose via fp8 matmuls
    against scaled identities into [128, 2, NPAD] fp8 residents;
    h2^T = relu(W1 AH^T + W2 A2H^T) (DoubleRow over the two hid halves) and
    g = softplus(W_out h2^T + b_out), one 512-col block at a time,
    interleaved into the last hop-2 pass. All fp8 scale factors are powers
    of two (exact).
"""

import sys

for _p in ("/opt/trn_rl_repo", "/root/.axon_site/_ro/trn_rl_repo"):
    if _p not in sys.path:
        sys.path.insert(0, _p)

import numpy as np
import ml_dtypes

import concourse.bacc as bacc
import concourse.bass as bass
import concourse.mybir as mybir
from concourse import tile
from concourse.bass_utils import run_bass_kernel_spmd

F32 = mybir.dt.float32
F32R = mybir.dt.float32r
BF16 = mybir.dt.bfloat16
FP8 = mybir.dt.float8e4
I16 = mybir.dt.int16
AF = mybir.ActivationFunctionType
DR = mybir.MatmulPerfMode.DoubleRow
NPFP8 = ml_dtypes.float8_e4m3fn

MAX_CALL_CHUNKS = 12      # <=12 chunks (1536 idxs) per dma_gather call
NQUEUES = 4               # SWDGE queues for gather concurrency
COLL_FRAC = 0.6           # emit next collective after this fraction of calls


class Cfg:
    def __init__(self, P, E, nc=8, hid=256):
        assert P % (nc * 2) == 0
        self.P, self.E, self.NC, self.HID = P, E, nc, hid
        self.NPC = P // nc                    # nodes per core
        self.NT = (self.NPC + 127) // 128     # dest tiles per core
        self.NPAD = self.NT * 128
        if self.NT > 40:
            self.SPLITS = [17, 34]            # bucket boundaries (tiles)
        else:
            self.SPLITS = [max(1, self.NT // 2)]
        bounds = [0] + self.SPLITS + [self.NT]
        self.NB = len(bounds) - 1
        self.BROWS = []                       # locals per bucket
        for i in range(self.NB):
            lo = bounds[i] * 128
            hi = min(bounds[i + 1] * 128, self.NPC)
            self.BROWS.append(hi - lo)
        self.BT = bounds                      # tile bounds per bucket
        self.BLK = []
        off = 0
        while off < self.NPAD:
            w = min(512, self.NPAD - off)
            self.BLK.append((off, w))
            off += w


def _plan(cfg, deg):
    """Hop-1 plan: chunk count per tile = max in-degree in the tile."""
    P, NC, NT = cfg.P, cfg.NC, cfg.NT
    order = np.argsort(-deg, kind="stable")
    rank = np.empty(P, np.int64)
    rank[order] = np.arange(P)
    core_of = rank % NC
    local_of = rank // NC
    gid = core_of * cfg.NPC + local_of
    degs_sorted = deg[order]
    NCHUNK = []
    for t in range(NT):
        NCHUNK.append(max(1, int(degs_sorted[min(t * 128 * NC, P - 1)])))
    NCHUNK = np.array(NCHUNK, np.int64)
    tile_off = np.concatenate([[0], np.cumsum(NCHUNK)])
    return core_of, local_of, gid, NCHUNK, tile_off, int(tile_off[-1])


def _split_calls(nchunks):
    """Split a chunk count into gather calls <= MAX_CALL_CHUNKS, keeping
    every non-final call even so DoubleRow pairs never straddle calls."""
    out = []
    rem = nchunks
    while rem:
        g = min(MAX_CALL_CHUNKS, rem)
        if g < rem and g % 2:
            g -= 1
        out.append(g)
        rem -= g
    return out


def _pack_pairs(x4T, lo, hi):
    """Pack chunks [lo,hi) of x4T ([4, TC*128]) in h-pair layout: pair p ->
    partitions 32*(p%4)+(0..8), col block p//4. Odd tail chunk packs alone
    in the A-half of its pair slot."""
    n = hi - lo
    npr = (n + 1) // 2
    NQ = (npr + 3) // 4
    x4q = np.zeros((128, NQ * 128), np.float32)
    for p in range(npr):
        j, q = p % 4, p // 4
        kA = lo + 2 * p
        x4q[32 * j:32 * j + 4, q * 128:(q + 1) * 128] = \
            x4T[:, kA * 128:(kA + 1) * 128]
        if 2 * p + 1 < n:
            kB = kA + 1
            x4q[32 * j + 4:32 * j + 8, q * 128:(q + 1) * 128] = \
                x4T[:, kB * 128:(kB + 1) * 128]
    return x4q, NQ


def _prepare(cfg, beta, degree, A_rows, A_cols, A_vals,
             W_in, b_in, W_mp1, W_mp2, W_out, b_out):
    P, E, NC, NPC, NT = cfg.P, cfg.E, cfg.NC, cfg.NPC, cfg.NT
    NB = cfg.NB
    deg = np.bincount(A_rows, minlength=P).astype(np.int64)
    core_of, local_of, gid, NCHUNK, tile_off, TC = _plan(cfg, deg)

    # ---- hop-1 edge slots (slot column == dest column) ----
    d_gid = gid[A_rows.astype(np.int64)]
    oe = np.argsort(d_gid, kind="stable")
    sd = d_gid[oe]
    first = np.r_[True, sd[1:] != sd[:-1]]
    cumstart = np.maximum.accumulate(np.where(first, np.arange(E), 0))
    chunk = np.arange(E) - cumstart
    e_core = sd // NPC
    e_local = sd % NPC
    e_col = e_local % 128
    e_k = tile_off[e_local // 128] + chunk
    e_slot = e_k * 128 + e_col
    src1 = A_cols.astype(np.int64)[oe]
    vals1 = A_vals[oe].astype(np.float32)

    x4_all = np.stack([beta[:, 0], beta[:, 0] ** 2, degree[:, 0],
                       np.ones(P, np.float32)], axis=0).astype(np.float32)

    # ---- hop-2 edge plan: sort by (core, tile, bucket) ----
    s_gid = gid[A_cols.astype(np.int64)]
    c2_core = d_gid // NPC
    c2_loc = d_gid % NPC
    c2_tile = c2_loc // 128
    c2_dcol = c2_loc % 128
    s_loc = s_gid % NPC
    s_core = s_gid // NPC
    blo = np.array([cfg.BT[i] * 128 for i in range(NB)], np.int64)
    c2_b = np.searchsorted(blo, s_loc, side="right") - 1
    brows = np.array(cfg.BROWS, np.int64)
    c2_tidx = s_core * brows[c2_b] + (s_loc - blo[c2_b])
    o2 = np.lexsort((c2_b, c2_tile, c2_core))
    g_core = c2_core[o2]
    g_tile = c2_tile[o2]
    g_b = c2_b[o2]
    g_dcol = c2_dcol[o2]
    g_tidx = c2_tidx[o2]
    g_val = A_vals[o2].astype(np.float32)
    key = (g_core * NT + g_tile) * NB + g_b
    kfirst = np.r_[True, key[1:] != key[:-1]]
    kcum = np.maximum.accumulate(np.where(kfirst, np.arange(E), 0))
    g_pos = np.arange(E) - kcum

    # shared SPMD structure (max over cores, min 1 chunk per (t,b))
    cnt_all = np.zeros((NC, NT, NB), np.int64)
    np.add.at(cnt_all, (g_core, g_tile, g_b), 1)
    nch = np.maximum(1, -(-cnt_all.max(axis=0) // 128))   # [NT, NB]
    flat = nch.reshape(-1)
    cbase = np.concatenate([[0], np.cumsum(flat)]).astype(np.int64)
    TOT = int(cbase[-1])
    # calls grouped bucket-major (pass order)
    calls = []          # (tile, bucket, chunk_base, g)
    for b in range(NB):
        for t in range(NT):
            base = int(cbase[t * NB + b])
            for g in _split_calls(int(nch[t, b])):
                calls.append((t, b, base, g))
                base += g
    NIC = sum(g * 128 // 16 for (_, _, _, g) in calls)

    part_bounds = [int(tile_off[bt]) for bt in cfg.BT]    # chunk bounds

    per_core = []
    for c in range(NC):
        # ---- hop 1 arrays ----
        m1 = e_core == c
        sl1 = e_slot[m1]
        x4T = np.zeros((4, TC * 128), np.float32)
        x4T[:, sl1] = x4_all[:, src1[m1]]
        v1 = np.zeros((128, TC), np.float32)
        v1[e_col[m1], e_k[m1]] = 16.0 * vals1[m1]
        xparts = []
        for i in range(NB):
            x4q, NQ = _pack_pairs(x4T, part_bounds[i], part_bounds[i + 1])
            xparts.append(x4q)

        # ---- hop 2 arrays ----
        m2 = g_core == c
        e_key = (g_tile[m2] * NB + g_b[m2])
        e_chunk = cbase[e_key] + g_pos[m2] // 128
        e_p = g_pos[m2] % 128
        S8 = np.zeros((128, TOT * 128), np.float32)
        S8[e_p, e_chunk * 128 + g_dcol[m2]] = 16.0 * g_val[m2]
        S8 = S8.astype(NPFP8)
        slot_idx = np.zeros(TOT * 128, np.int64)
        slot_idx[e_chunk * 128 + e_p] = g_tidx[m2]

        idxh = np.zeros((128, NIC), np.int16)
        col0 = 0
        for (t, b, base, g) in calls:
            ni = g * 128
            blockv = slot_idx[base * 128:base * 128 + ni].astype(np.int16)
            blockv = blockv.reshape(ni // 16, 16).T
            for q in range(8):
                idxh[16 * q:16 * (q + 1), col0:col0 + ni // 16] = blockv
            col0 += ni // 16
        pc = dict(v1=v1, s8=S8, idx=idxh)
        for i in range(NB):
            pc[f"x4_{i}"] = xparts[i]
        per_core.append(pc)

    # ---- constants (power-of-two scaled for fp8) ----
    wiT = np.concatenate([W_in.T.astype(np.float32),
                          b_in[None, :].astype(np.float32)], axis=0)
    HID = cfg.HID
    wiT2 = np.zeros((128, 2 * HID), np.float32)
    for j in range(4):
        wiT2[32 * j:32 * j + 4, 0:HID] = wiT
        wiT2[32 * j + 4:32 * j + 8, HID:2 * HID] = wiT

    def pack_w(W, scale):
        w = (scale * W.T.astype(np.float32)).reshape(2, 128, W.shape[0])
        return np.ascontiguousarray(w.transpose(1, 0, 2)).astype(NPFP8)

    idn2 = np.zeros((128, 2, 128), np.float32)
    idn2[np.arange(128), 0, np.arange(128)] = 1.0
    idn2[np.arange(128), 1, np.arange(128)] = 1.0

    consts = dict(
        wit2=wiT2,
        w1s=pack_w(W_mp1, 32.0),
        w2s=pack_w(W_mp2, 16.0),
        wos=(16.0 * W_out.reshape(2, 128).T.reshape(128, 2, 1)
             .astype(np.float32)).astype(NPFP8),
        bout=np.full((128, 1), float(np.asarray(b_out).reshape(-1)[0]),
                     np.float32),
        idn2=idn2.astype(NPFP8),
        sidn8=(np.eye(128, dtype=np.float32) * 0.125).astype(NPFP8),
        sidn4=(np.eye(128, dtype=np.float32) * 0.25).astype(NPFP8),
    )
    meta = dict(NCHUNK=tuple(int(x) for x in NCHUNK), TC=TC,
                nch=tuple(int(x) for r in nch for x in r),
                calls=tuple(calls), TOT=TOT, NIC=NIC,
                NQs=tuple((((part_bounds[i + 1] - part_bounds[i]) + 1) // 2
                           + 3) // 4 for i in range(NB)))
    return per_core, consts, meta, (core_of, local_of)


def _build(cfg, meta):
    NT, NPC, NPAD, HID, NC, P, NB = (cfg.NT, cfg.NPC, cfg.NPAD, cfg.HID,
                                     cfg.NC, cfg.P, cfg.NB)
    NCHUNK = meta["NCHUNK"]
    TC, NIC, TOT = meta["TC"], meta["NIC"], meta["TOT"]
    calls = meta["calls"]
    nch = np.array(meta["nch"], np.int64).reshape(NT, NB)
    tile_off = np.concatenate([[0], np.cumsum(NCHUNK)])
    NBLK = len(cfg.BLK)
    NQs = meta["NQs"]
    part_bounds = [int(tile_off[bt]) for bt in cfg.BT]

    nc = bacc.Bacc("TRN2", target_bir_lowering=False, debug=False,
                   num_swdge_queues=NQUEUES)
    x4_d = [nc.dram_tensor(f"x4_{i}", [128, max(NQs[i], 1) * 128], F32R,
                           kind="ExternalInput") for i in range(NB)]
    v1_d = nc.dram_tensor("v1", [128, TC], F32, kind="ExternalInput")
    s8_d = nc.dram_tensor("s8", [128, TOT * 128], FP8, kind="ExternalInput")
    idx_d = nc.dram_tensor("idx", [128, NIC], I16, kind="ExternalInput")
    wiT2_d = nc.dram_tensor("wit2", [128, 2 * HID], F32R,
                            kind="ExternalInput")
    w1s_d = nc.dram_tensor("w1s", [128, 2 * HID], FP8, kind="ExternalInput")
    w2s_d = nc.dram_tensor("w2s", [128, 2 * HID], FP8, kind="ExternalInput")
    wos_d = nc.dram_tensor("wos", [128, 2], FP8, kind="ExternalInput")
    bout_d = nc.dram_tensor("bout", [128, 1], F32, kind="ExternalInput")
    idn2_d = nc.dram_tensor("idn2", [128, 2 * 128], FP8, kind="ExternalInput")
    sidn8_d = nc.dram_tensor("sidn8", [128, 128], FP8, kind="ExternalInput")
    sidn4_d = nc.dram_tensor("sidn4", [128, 128], FP8, kind="ExternalInput")
    g_d = nc.dram_tensor("g", [1, NBLK * 512], F32, kind="ExternalOutput")

    bounce = [nc.dram_tensor(f"bounce{i}", [cfg.BROWS[i], HID], FP8)
              for i in range(NB)]
    table = [nc.dram_tensor(f"table{i}", [NC * cfg.BROWS[i], HID], FP8,
                            addr_space="Shared") for i in range(NB)]

    with tile.TileContext(nc) as tc:
        with (
            tc.tile_pool(name="const", bufs=1) as constp,
            tc.tile_pool(name="xs", bufs=3) as xsp,
            tc.tile_pool(name="msgs", bufs=6) as msgp,
            tc.tile_pool(name="sd", bufs=8) as sdp,
            tc.tile_pool(name="stage", bufs=3) as stagep,
            tc.tile_pool(name="resid", bufs=1) as residp,
            tc.tile_pool(name="pair", bufs=24) as pairp,
            tc.tile_pool(name="ph", bufs=2, space="PSUM") as php,
            tc.tile_pool(name="pz", bufs=2, space="PSUM") as pzp,
            tc.tile_pool(name="pt", bufs=2, space="PSUM") as ptp,
        ):
            wiT2 = constp.tile([128, 2 * HID], F32R, tag="wiT2", name="wiT2")
            nc.sync.dma_start(wiT2[:], wiT2_d[:])
            w1s = constp.tile([128, 2, HID], FP8, tag="w1s", name="w1s")
            nc.sync.dma_start(w1s[:], w1s_d[:])
            w2s = constp.tile([128, 2, HID], FP8, tag="w2s", name="w2s")
            nc.sync.dma_start(w2s[:], w2s_d[:])
            wos = constp.tile([128, 2, 1], FP8, tag="wos", name="wos")
            nc.sync.dma_start(wos[:], wos_d[:])
            bout = constp.tile([128, 1], F32, tag="bout", name="bout")
            nc.sync.dma_start(bout[:], bout_d[:])
            idn2 = constp.tile([128, 2, 128], FP8, tag="idn2", name="idn2")
            nc.sync.dma_start(idn2[:], idn2_d[:])
            sidn8 = constp.tile([128, 128], FP8, tag="sidn8", name="sidn8")
            nc.sync.dma_start(sidn8[:], sidn8_d[:])
            sidn4 = constp.tile([128, 128], FP8, tag="sidn4", name="sidn4")
            nc.sync.dma_start(sidn4[:], sidn4_d[:])
            v1 = constp.tile([128, TC], F32, tag="v1", name="v1")
            nc.sync.dma_start(v1[:], v1_d[:])
            idx = constp.tile([128, NIC], I16, tag="idx", name="idx")
            nc.sync.dma_start(idx[:], idx_d[:])

            ahT = residp.tile([128, 2, NPAD], FP8, tag="ahT", name="ahT")
            a2T = residp.tile([128, 2, NPAD], FP8, tag="a2T", name="a2T")
            partial = residp.tile([128, NT, HID], BF16, tag="part",
                                  name="part")

            # ---- phase A: hop 1 ------------------------------------------
            def epilogue_a(t, pz, used_right, part_i):
                ahb = stagep.tile([128, HID], FP8, tag="ahb", name="ahb")
                if used_right:
                    rh = stagep.tile([128, HID], BF16, tag="rh", name="rh")
                    nc.scalar.activation(rh[:], pz[:, HID:2 * HID], AF.Copy)
                    nc.vector.tensor_tensor(
                        ahb[:], pz[:, :HID], rh[:],
                        op=mybir.AluOpType.add)
                else:
                    nc.scalar.activation(ahb[:], pz[:, :HID], AF.Copy)
                r0 = t * 128 - cfg.BT[part_i] * 128
                rows = min(128, NPC - t * 128)
                nc.sync.dma_start(bounce[part_i][r0:r0 + rows, :],
                                  ahb[:rows, :])
                for mh in (0, 1):
                    pt = ptp.tile([128, 512], F32, tag="pt", name="pt")
                    nc.tensor.matmul(
                        pt[:, :128], lhsT=ahb[:, mh * 128:(mh + 1) * 128],
                        rhs=sidn8[:], start=True, stop=True,
                        skip_group_check=True)
                    nc.vector.tensor_copy(
                        ahT[:, mh, t * 128:(t + 1) * 128], pt[:, :128])

            def phase_a(part_i):
                lo, hi = part_bounds[part_i], part_bounds[part_i + 1]
                xd = x4_d[part_i]
                NQp = NQs[part_i]
                t = int(np.searchsorted(tile_off, lo, side="right")) - 1
                pz = None
                mq = None          # quad message tile [128, 2, 512]
                mq2 = None         # leftover pair tile [128, 2, 256]
                xs = None
                for p in range((hi - lo + 1) // 2):
                    if p % 4 == 0:
                        xs = xsp.tile([128, 128], F32R, tag="xs", name="xs")
                        q = p // 4
                        nc.sync.dma_start(xs[:],
                                          xd[:, q * 128:(q + 1) * 128])
                    j = p % 4
                    kA = lo + 2 * p
                    single = kA + 1 >= hi
                    ph = php.tile([128, 512], F32, tag="ph", name="ph",
                                  bufs=4)
                    nc.tensor.matmul(
                        ph[:, :2 * HID],
                        lhsT=xs[32 * j:32 * j + 8, :],
                        rhs=wiT2[32 * j:32 * j + 8, :],
                        start=True, stop=True, skip_group_check=True,
                        tile_position=(32 * j, 0))
                    for k in (kA,) if single else (kA, kA + 1):
                        if k == int(tile_off[t]):
                            pz = pzp.tile([128, 512], F32, tag="acc",
                                          name="acc")
                        nchk = int(NCHUNK[t])
                        q_in = k - int(tile_off[t])
                        nq = nchk // 4
                        tstart = q_in == 0
                        tlast = q_in == nchk - 1
                        ph_half = ph[:, (k - kA) * HID:(k - kA + 1) * HID]
                        # destination quarter for this chunk's message
                        if q_in < 4 * nq:
                            qq = q_in % 4
                            if qq == 0:
                                mq = msgp.tile([128, 2, 2 * HID], FP8,
                                               tag="mq", name="mq")
                            dst = mq[:, qq % 2, (qq // 2) * HID:
                                     (qq // 2 + 1) * HID]
                        else:
                            rr = q_in - 4 * nq
                            if rr == 0:
                                mq2 = msgp.tile([128, 2, HID], FP8,
                                                tag="mq2", name="mq2")
                            dst = mq2[:, rr % 2, :] if rr < 2 \
                                else mq2[:, 0, :]
                            if rr == 2:
                                mq2 = msgp.tile([128, 2, HID], FP8,
                                                tag="mq2", name="mq2")
                                dst = mq2[:, 0, :]
                        if k % 2 == 0:
                            nc.scalar.activation(dst, ph_half, AF.Relu,
                                                 scale=v1[:, k:k + 1])
                        else:
                            nc.vector.tensor_scalar(
                                dst, ph_half, v1[:, k:k + 1], 0.0,
                                op0=mybir.AluOpType.mult,
                                op1=mybir.AluOpType.max)
                        # emit accumulation matmuls
                        if q_in < 4 * nq and q_in % 4 == 3:
                            nc.tensor.matmul(
                                pz[:, :2 * HID], lhsT=idn2[:], rhs=mq[:],
                                perf_mode=DR, start=(q_in == 3),
                                stop=tlast, skip_group_check=True)
                        elif q_in >= 4 * nq:
                            rr = q_in - 4 * nq
                            rem = nchk - 4 * nq
                            if rr == 1 and rem >= 2:
                                nc.tensor.matmul(
                                    pz[:, :HID], lhsT=idn2[:], rhs=mq2[:],
                                    perf_mode=DR, start=(nq == 0 and rr == 1),
                                    stop=(q_in == nchk - 1),
                                    skip_group_check=True)
                            elif rr == 0 and rem == 1 or rr == 2:
                                nc.tensor.matmul(
                                    pz[:, :HID], lhsT=idn2[:, 0, :],
                                    rhs=mq2[:, 0, :],
                                    start=(nq == 0 and rr == 0),
                                    stop=tlast, skip_group_check=True)
                        if tlast:
                            epilogue_a(t, pz, nq > 0, part_i)
                            t += 1

            # ---- emit phase A parts + collectives + phase C passes -------
            for i in range(NB):
                phase_a(i)

            def collective(i):
                nc.gpsimd.collective_compute(
                    "AllGather", mybir.AluOpType.bypass,
                    replica_groups=[list(range(NC))],
                    ins=[bounce[i].ap().opt()],
                    outs=[table[i].ap().opt()],
                )

            collective(0)

            def dense_block(bidx):
                off, w = cfg.BLK[bidx]
                ht = stagep.tile([128, 2, 512], FP8, tag="h2t", name="h2t")
                for mh in (0, 1):
                    pd = pzp.tile([128, 512], F32, tag="acc", name="acc")
                    nc.tensor.matmul(
                        pd[:, :w], lhsT=w1s[:, :, mh * 128:(mh + 1) * 128],
                        rhs=ahT[:, :, off:off + w], perf_mode=DR,
                        start=True, stop=False, skip_group_check=True)
                    nc.tensor.matmul(
                        pd[:, :w], lhsT=w2s[:, :, mh * 128:(mh + 1) * 128],
                        rhs=a2T[:, :, off:off + w], perf_mode=DR,
                        start=False, stop=True, skip_group_check=True)
                    nc.scalar.activation(ht[:, mh, :w], pd[:, :w], AF.Relu,
                                         scale=0.015625)
                pg = ptp.tile([1, 512], F32, tag="pt", name="pg")
                for i in (0, 1):
                    nc.tensor.matmul(pg[:, :w], lhsT=wos[:, i, :],
                                     rhs=ht[:, i, :w],
                                     start=(i == 0), stop=(i == 1),
                                     skip_group_check=True)
                gb = stagep.tile([1, 512], F32, tag="gbuf", name="gb",
                                 bufs=4)
                nc.vector.tensor_copy(gb[0:1, :w], pg[:, :w])
                ge = stagep.tile([1, 512], F32, tag="gbuf", name="ge",
                                 bufs=4)
                nc.scalar.activation(ge[0:1, :w], gb[0:1, :w], AF.Exp,
                                     bias=bout[0:1, :], scale=0.0625)
                go = stagep.tile([1, 512], F32, tag="gbuf", name="go",
                                 bufs=4)
                nc.scalar.activation(go[0:1, :w], ge[0:1, :w], AF.Ln,
                                     bias=1.0)
                nc.sync.dma_start(g_d[0:1, off:off + w], go[0:1, :w])

            # phase C: one pass per bucket
            ci = 0
            col0 = 0
            qrr = 0
            for b in range(NB):
                bcalls = [cl for cl in calls if cl[1] == b]
                ncoll = max(1, int(COLL_FRAC * len(bcalls)))
                nc_done = 0
                for t in range(NT):
                    ncht = int(nch[t, b])
                    pz = pzp.tile([128, 512], F32, tag="acc", name="acc")
                    done = 0
                    while done < ncht:
                        (tt, bb, base, g) = calls[ci]
                        assert tt == t and bb == b
                        ni = g * 128
                        pr = pairp.tile([128, MAX_CALL_CHUNKS, HID], FP8,
                                        tag="pair", name="pair")
                        nc.gpsimd.dma_gather(
                            pr[:, :g, :], table[b].ap(),
                            idx[:, col0:col0 + ni // 16],
                            ni, ni, HID, single_packet=False,
                            queue_num=qrr)
                        qrr = (qrr + 1) % NQUEUES
                        sd = sdp.tile([128, MAX_CALL_CHUNKS, 128], FP8,
                                      tag="sdl", name="sdl")
                        nc.sync.dma_start(
                            sd[:, :g, :],
                            s8_d[:, base * 128:(base + g) * 128])
                        for cc in range(0, g - 1, 2):
                            nc.tensor.matmul(
                                pz[:, :HID], lhsT=sd[:, cc:cc + 2, :],
                                rhs=pr[:, cc:cc + 2, :], perf_mode=DR,
                                start=(done + cc == 0),
                                stop=(done + cc + 2 == ncht),
                                skip_group_check=True)
                        if g % 2:
                            nc.tensor.matmul(
                                pz[:, :HID], lhsT=sd[:, g - 1, :],
                                rhs=pr[:, g - 1, :],
                                start=(done + g - 1 == 0),
                                stop=(done + g == ncht),
                                skip_group_check=True)
                        done += g
                        col0 += ni // 16
                        ci += 1
                        nc_done += 1
                        if b + 1 < NB and nc_done == ncoll:
                            collective(b + 1)
                    # combine into partial / final epilogue
                    if b + 1 < NB:
                        if b == 0:
                            nc.vector.tensor_scalar(
                                partial[:, t, :], pz[:, :HID], 0.0625, 0.0,
                                op0=mybir.AluOpType.mult,
                                op1=mybir.AluOpType.bypass)
                        else:
                            t1 = stagep.tile([128, HID], BF16, tag="t1",
                                             name="t1")
                            nc.vector.tensor_scalar(
                                t1[:], pz[:, :HID], 0.0625, 0.0,
                                op0=mybir.AluOpType.mult,
                                op1=mybir.AluOpType.bypass)
                            nc.vector.tensor_tensor(
                                partial[:, t, :], partial[:, t, :], t1[:],
                                op=mybir.AluOpType.add)
                    else:
                        t1 = stagep.tile([128, HID], BF16, tag="t1",
                                         name="t1")
                        nc.vector.tensor_scalar(
                            t1[:], pz[:, :HID], 0.0625, 0.0,
                            op0=mybir.AluOpType.mult,
                            op1=mybir.AluOpType.bypass)
                        a2b = stagep.tile([128, HID], FP8, tag="a2b",
                                          name="a2b")
                        nc.vector.tensor_tensor(
                            a2b[:], partial[:, t, :], t1[:],
                            op=mybir.AluOpType.add)
                        for mh in (0, 1):
                            pt = ptp.tile([128, 512], F32, tag="pt",
                                          name="pt")
                            nc.tensor.matmul(
                                pt[:, :128],
                                lhsT=a2b[:, mh * 128:(mh + 1) * 128],
                                rhs=sidn4[:], start=True, stop=True,
                                skip_group_check=True)
                            nc.vector.tensor_copy(
                                a2T[:, mh, t * 128:(t + 1) * 128],
                                pt[:, :128])
                        if t % 4 == 3:
                            dense_block(t // 4)
            for bidx in range(NT // 4, NBLK):
                dense_block(bidx)

    nc.compile()
    return nc


_COMPILED = {}


def _get_compiled(cfg, meta):
    key = (cfg.P, cfg.E, meta["NCHUNK"], meta["nch"], meta["calls"])
    if key not in _COMPILED:
        _COMPILED[key] = _build(cfg, meta)
    return _COMPILED[key]


def run(cfg, inputs, trace=False):
    per_core, consts, meta, (core_of, local_of) = _prepare(cfg, **inputs)
    ncobj = _get_compiled(cfg, meta)
    in_maps = []
    for c in range(cfg.NC):
        im = dict(per_core[c])
        im.update({k: np.asarray(v) for k, v in consts.items()})
        in_maps.append(im)
    res = run_bass_kernel_spmd(ncobj, in_maps, list(range(cfg.NC)),
                               trace=trace)
    g = np.empty(cfg.P, np.float32)
    for c in range(cfg.NC):
        go = np.asarray(res.results[c]["g"]).reshape(-1)
        mine = core_of == c
        g[mine] = go[local_of[mine]]
    return g.reshape(cfg.P, 1), res


def kernel(**inputs):
    cfg = Cfg(P=50000, E=800000)
    g, _ = run(cfg, inputs)
    return g
